# revision 33
# baseline (speedup 1.0000x reference)
"""Causal self-attention (B=4, S=2048, E=1024, H=16) on 8 trn2 NeuronCores.

Sharding: data parallel over batch (4) x tensor parallel over head groups (2).
Core c handles batch c//2, heads [ (c%2)*8, (c%2)*8+8 ).  Each core computes
its group's QKV projections, causal attention, and a partial output
projection; the host sums the two group partials per batch and adds bo.

All matmuls run in float32r (fp32 storage, single-pass relaxed-precision PE
mode — full bf16-rate at free dim >= 256, ~2^-10 product precision), with
fp32 PSUM accumulation.  Measured end-to-end error is ~10x tighter than a
bf16 datapath.

Device layout notes:
  - x arrives pre-transposed from host: xT [E, S], streamed per 512-token
    chunk.  QT/KT are produced feature-major [c, s] (lhsT = W, rhs = xT);
    V token-major [s, f] (lhsT = xT, rhs = Wv).
  - scores are computed transposed [k, q]: lhsT = KT head slice [64, 128],
    rhs = QT head slice [64, 512]; head parity selects partition base 0/64.
  - softmax denominator comes from a 65th ones-column in the AV stationary
    operand: PSUM row 64 of the [65, q] attention-value output is the
    running row-sum of exp(scores).  Its reciprocal (VectorE mid-pass,
    exp(-ln d) on ScalarE for latency-critical pass tails) is broadcast to
    64 partitions via a DRAM-bounce DMA.
  - causal masking: fully-masked k-tiles are skipped; on diagonal tiles the
    dead columns are zeroed on VectorE and the 128-wide triangle multiplied
    in on GpSimd, all off the PE critical path (AV matmuls lag 4 k-tiles).
"""

import numpy as np

import concourse.mybir as mybir
import concourse.tile as tile
from concourse import bacc
from concourse.bass_utils import run_bass_kernel_spmd

F32 = mybir.dt.float32
F32R = mybir.dt.float32r
Exp = mybir.ActivationFunctionType.Exp
MULT = mybir.AluOpType.mult
ADD = mybir.AluOpType.add

B, S, E, H = 4, 2048, 1024, 16
D = 64          # head dim
HG = 8          # heads per core
G = 512         # group feature width
P = 128
NKT = S // P    # 16 k-tiles
NST = S // P    # 16 s-tiles
QB = 512        # q-block width
NQB = S // QB   # 4
ESUB = E // P   # 8
VW = D + 1      # V stationary width (64 dims + ones column)

_CACHE = {}


def _build_program():
    nc = bacc.Bacc("TRN2", target_bir_lowering=False, debug=False)

    xt_d = nc.dram_tensor("xt", [E, S], F32R, kind="ExternalInput").ap()
    wq_d = nc.dram_tensor("wq", [E, G], F32R, kind="ExternalInput").ap()
    wk_d = nc.dram_tensor("wk", [E, G], F32R, kind="ExternalInput").ap()
    wv_d = nc.dram_tensor("wv", [E, G], F32R, kind="ExternalInput").ap()
    wo_d = nc.dram_tensor("wo", [G, E], F32R, kind="ExternalInput").ap()
    bq_d = nc.dram_tensor("bq", [P, 4], F32, kind="ExternalInput").ap()
    bk_d = nc.dram_tensor("bk", [P, 4], F32, kind="ExternalInput").ap()
    bv_d = nc.dram_tensor("bv", [P, G], F32, kind="ExternalInput").ap()
    tri_d = nc.dram_tensor("tri", [P, P], F32R, kind="ExternalInput").ap()
    one_d = nc.dram_tensor("one", [P, D], F32R, kind="ExternalInput").ap()
    out_d = nc.dram_tensor("out", [S, E], F32, kind="ExternalOutput").ap()
    # scratch for the reciprocal-row broadcast bounce
    rc_d = nc.dram_tensor("rc_scratch", [HG, NQB, QB], F32, kind="Internal").ap()

    qt_sb = nc.alloc_sbuf_tensor("qt_sb", [P, 4, S], F32R).ap()
    kt_sb = nc.alloc_sbuf_tensor("kt_sb", [P, 4, S], F32R).ap()
    vx_sb = nc.alloc_sbuf_tensor("vx_sb", [P, NKT, HG, VW], F32R).ap()
    tri_sb = nc.alloc_sbuf_tensor("tri_sb", [P, P], F32R).ap()
    ones_sb = nc.alloc_sbuf_tensor("ones_sb", [P, D], F32R).ap()
    bq_sb = nc.alloc_sbuf_tensor("bq_sb", [P, 4], F32).ap()
    bk_sb = nc.alloc_sbuf_tensor("bk_sb", [P, 4], F32).ap()
    bv_sb = nc.alloc_sbuf_tensor("bv_sb", [P, G], F32).ap()

    with tile.TileContext(nc) as tc:
        nc.sync.dma_start(bq_sb[:], bq_d[:])
        nc.sync.dma_start(bk_sb[:], bk_d[:])
        nc.sync.dma_start(bv_sb[:], bv_d[:])
        nc.sync.dma_start(tri_sb[:], tri_d[:])
        nc.sync.dma_start(ones_sb[:], one_d[:])

        xt_r = xt_d.rearrange("(o p) s -> p o s", p=P)

        # ---- QKV projections (xT streamed per 512-token chunk) ----
        with (
            tc.tile_pool(name="w_pool", bufs=1) as wp,
            tc.tile_pool(name="xt_pool", bufs=2) as xp,
            tc.tile_pool(name="proj_ps", bufs=2, space="PSUM") as pp,
        ):
            wqp = wp.tile([P, ESUB, G], F32R, tag="wq", name="wqp")
            wkp = wp.tile([P, ESUB, G], F32R, tag="wk", name="wkp")
            wvp = wp.tile([P, ESUB, G], F32R, tag="wv", name="wvp")
            # first xT chunk loads ahead of the weights so the PE starts sooner
            xtp0 = xp.tile([P, ESUB, QB], F32R, tag="xt", name="xtp")
            nc.sync.dma_start(xtp0[:], xt_r[:, :, 0:QB])
            for e in range(ESUB):
                nc.sync.dma_start(
                    wqp[:, e, :], wq_d.rearrange("(o p) f -> p o f", p=P)[:, e, :]
                )
                nc.sync.dma_start(
                    wkp[:, e, :], wk_d.rearrange("(o p) f -> p o f", p=P)[:, e, :]
                )
                nc.sync.dma_start(
                    wvp[:, e, :], wv_d.rearrange("(o p) f -> p o f", p=P)[:, e, :]
                )
            for qb in range(NQB):
                if qb == 0:
                    xtp = xtp0
                else:
                    xtp = xp.tile([P, ESUB, QB], F32R, tag="xt", name="xtp")
                    nc.sync.dma_start(xtp[:], xt_r[:, :, qb * QB : (qb + 1) * QB])
                for cc in range(4):
                    q_ps = pp.tile([P, QB], F32, tag="q", name="q_ps")
                    for e in range(ESUB):
                        nc.tensor.matmul(
                            q_ps[:],
                            lhsT=wqp[:, e, cc * P : (cc + 1) * P],
                            rhs=xtp[:, e, :],
                            start=(e == 0),
                            stop=(e == ESUB - 1),
                        )
                    # fold bias and the 1/sqrt(D) scale into Q
                    nc.vector.tensor_scalar(
                        qt_sb[:, cc, qb * QB : (qb + 1) * QB],
                        q_ps[:],
                        bq_sb[:, cc : cc + 1],
                        0.125,
                        ADD,
                        MULT,
                    )
                    k_ps = pp.tile([P, QB], F32, tag="k", name="k_ps")
                    for e in range(ESUB):
                        nc.tensor.matmul(
                            k_ps[:],
                            lhsT=wkp[:, e, cc * P : (cc + 1) * P],
                            rhs=xtp[:, e, :],
                            start=(e == 0),
                            stop=(e == ESUB - 1),
                        )
                    nc.vector.tensor_scalar_add(
                        kt_sb[:, cc, qb * QB : (qb + 1) * QB],
                        k_ps[:],
                        bk_sb[:, cc : cc + 1],
                    )
                for stl in range(4):
                    st = qb * 4 + stl
                    v_ps = pp.tile([P, G], F32, tag="v", name="v_ps")
                    for e in range(ESUB):
                        nc.tensor.matmul(
                            v_ps[:],
                            lhsT=xtp[:, e, stl * P : (stl + 1) * P],
                            rhs=wvp[:, e, :],
                            start=(e == 0),
                            stop=(e == ESUB - 1),
                        )
                    nc.vector.tensor_tensor(
                        vx_sb[:, st, :, 0:D],
                        v_ps.rearrange("p (h d) -> p h d", d=D),
                        bv_sb.rearrange("p (h d) -> p h d", d=D),
                        ADD,
                    )
                    nc.vector.tensor_copy(
                        vx_sb[:, st, :, D : D + 1],
                        ones_sb[:, 0:HG].rearrange("p (h u) -> p h u", u=1),
                    )

        # ---- attention + output projection ----
        with tc.tile_pool(name="at_pool", bufs=1) as atp:
            at_t = atp.tile([P, 4, S], F32R, name="at_t")
            # wo lives in the same long-lived pool and loads during attention
            wop = atp.tile([P, 4, E], F32R, name="wop")
            nc.sync.dma_start(wop[:], wo_d.rearrange("(o p) n -> p o n", p=P))
            with (
                tc.tile_pool(name="attn_ps", bufs=1, space="PSUM") as ap,
                tc.tile_pool(name="attn_sb", bufs=2) as sp,
            ):
                # heads run in even/odd pairs: the pair's score matmuls use
                # partition bases 0/64 (distinct PE row groups -> concurrent),
                # and the pair keeps the PE fed while ScalarE runs exp.
                # AV matmuls lag the scores by AV_LAG k-tiles so the PE never
                # waits on exp/mask; accumulators are per-(head, q-block) so
                # normalization runs mid-pass and frees PSUM slots early.
                AV_LAG = 4
                for pair in range(4):
                    heads = (2 * pair, 2 * pair + 1)
                    sub = pair

                    def normalize(h, qb):
                        hb = (h % 2) * D
                        a_ps = a_t[(h, qb)]
                        rs = sp.tile([VW, QB], F32, tag="rs", name="rs", bufs=2)
                        nc.vector.reciprocal(rs[D:VW, :], a_ps[D:VW, :])
                        # broadcast the reciprocal row to 64 partitions via a
                        # DRAM bounce (a DRAM source AP may repeat, SBUF cannot)
                        rb_sb = sp.tile([D, QB], F32, tag="rbs", name="rb_sb", bufs=2)
                        nc.sync.dma_start(rc_d[h, qb : qb + 1, :], rs[D:VW, :])
                        nc.sync.dma_start(
                            rb_sb[:], rc_d[h, qb : qb + 1, :].to_broadcast([D, QB])
                        )
                        at_slice = at_t[hb : hb + D, sub, qb * QB : (qb + 1) * QB]
                        if hb == 0:
                            nc.vector.tensor_tensor(
                                at_slice, a_ps[0:D, :], rb_sb[:], MULT
                            )
                        else:
                            tmp = sp.tile([D, QB], F32R, tag="tmp", name="tmp", bufs=2)
                            nc.vector.tensor_tensor(tmp[:], a_ps[0:D, :], rb_sb[:], MULT)
                            nc.sync.dma_start(at_slice, tmp[:])

                    for qp in range(2):
                        qb_lo, qb_hi = 2 * qp, 2 * qp + 1
                        a_t = {
                            (h, qb): ap.tile([VW, QB], F32, tag="a", name="a_ps", bufs=4)
                            for h in heads
                            for qb in (qb_lo, qb_hi)
                        }

                        def av_mms(pts_prev, kt_prev, qlo_prev):
                            for qb in range(qlo_prev, qb_hi + 1):
                                for h in heads:
                                    nc.tensor.matmul(
                                        a_t[(h, qb)][:],
                                        lhsT=vx_sb[:, kt_prev, h, :],
                                        rhs=pts_prev[qb][:, h % 2, :],
                                        start=(kt_prev == 0),
                                        stop=(kt_prev == 4 * qb + 3),
                                    )
                            for qb in range(qlo_prev, qb_hi + 1):
                                if kt_prev == 4 * qb + 3:
                                    for h in heads:
                                        normalize(h, qb)

                        pending = []
                        for kt in range(4 * qb_hi + 4):
                            qlo = max(qb_lo, kt // 4)
                            pts = {}
                            for qb in range(qlo, qb_hi + 1):
                                # one PSUM tile per (kt, qb): bank 0 = even
                                # head, bank 1 = odd head
                                s_t = ap.tile(
                                    [P, 2, QB], F32, tag="s", name="s_ps", bufs=2
                                )
                                for h in heads:
                                    hb = (h % 2) * D
                                    nc.tensor.matmul(
                                        s_t[:, h % 2, :],
                                        lhsT=kt_sb[
                                            hb : hb + D, sub, kt * P : (kt + 1) * P
                                        ],
                                        rhs=qt_sb[
                                            hb : hb + D, sub, qb * QB : (qb + 1) * QB
                                        ],
                                        start=True,
                                        stop=True,
                                    )
                                pt = sp.tile(
                                    [P, 2, QB], F32R, tag="pt", name="pt", bufs=9
                                )
                                nc.scalar.activation(pt[:], s_t[:], Exp)
                                if kt // 4 == qb:
                                    m = kt % 4
                                    for h in heads:
                                        if m > 0:
                                            nc.gpsimd.tensor_scalar_mul(
                                                pt[:, h % 2, 0 : P * m],
                                                pt[:, h % 2, 0 : P * m],
                                                0.0,
                                            )
                                        nc.gpsimd.tensor_tensor(
                                            pt[:, h % 2, P * m : P * (m + 1)],
                                            pt[:, h % 2, P * m : P * (m + 1)],
                                            tri_sb[:],
                                            MULT,
                                        )
                                pts[qb] = pt
                            if len(pending) >= AV_LAG:
                                av_mms(*pending.pop(0))
                            pending.append((pts, kt, qlo))
                        for args in pending:
                            av_mms(*args)

            # ---- output projection (partial; host adds other group + bo) ----
            with (
                tc.tile_pool(name="op_ps", bufs=2, space="PSUM") as op,
                tc.tile_pool(name="op_sb", bufs=3) as osp,
            ):
                for st in range(NST):
                    for n in range(2):
                        o_ps = op.tile([P, QB], F32, tag="o", name="o_ps")
                        for t in range(4):
                            nc.tensor.matmul(
                                o_ps[:],
                                lhsT=at_t[:, t, st * P : (st + 1) * P],
                                rhs=wop[:, t, n * QB : (n + 1) * QB],
                                start=(t == 0),
                                stop=(t == 3),
                            )
                        o_sb = osp.tile([P, QB], F32, tag="ost", name="o_sb")
                        nc.vector.tensor_copy(o_sb[:], o_ps[:])
                        nc.sync.dma_start(
                            out_d[st * P : (st + 1) * P, n * QB : (n + 1) * QB],
                            o_sb[:],
                        )

    nc.compile()
    return nc


def _prep_inputs(x, Wqkv, bqkv, Wo, bo):
    x = np.asarray(x, np.float32)
    Wqkv = np.asarray(Wqkv, np.float32)
    bqkv = np.asarray(bqkv, np.float32)
    Wo = np.asarray(Wo, np.float32)

    # 128x128 inclusive lower-triangle-in-(q,k) == kl <= ql in [k, q] layout
    kl = np.arange(P)[:, None]
    tri = (kl <= np.arange(P)[None, :]).astype(np.float32)


    in_maps = []
    for c in range(8):
        b, g = divmod(c, 2)
        lo, hi = g * G, (g + 1) * G
        in_maps.append(
            {
                "xt": np.ascontiguousarray(x[b].T),
                "wq": np.ascontiguousarray(Wqkv[:, lo:hi]),
                "wk": np.ascontiguousarray(Wqkv[:, E + lo : E + hi]),
                "wv": np.ascontiguousarray(Wqkv[:, 2 * E + lo : 2 * E + hi]),
                "wo": np.ascontiguousarray(Wo[lo:hi, :]),
                "bq": np.ascontiguousarray(bqkv[lo:hi].reshape(4, P).T),
                "bk": np.ascontiguousarray(bqkv[E + lo : E + hi].reshape(4, P).T),
                "bv": np.tile(bqkv[2 * E + lo : 2 * E + hi][None, :], (P, 1)).astype(
                    np.float32
                ),
                "tri": tri,
                "one": np.ones((P, D), np.float32),
            }
        )
    return in_maps


def kernel(x, Wqkv, bqkv, Wo, bo, _trace=False):
    if "nc" not in _CACHE:
        _CACHE["nc"] = _build_program()
    nc = _CACHE["nc"]

    in_maps = _prep_inputs(x, Wqkv, bqkv, Wo, bo)
    res = run_bass_kernel_spmd(nc, in_maps, core_ids=list(range(8)), trace=_trace)
    _CACHE["last_result"] = res

    bo = np.asarray(bo, np.float32)
    out = np.empty((B, S, E), np.float32)
    for b in range(B):
        out[b] = res.results[2 * b]["out"] + res.results[2 * b + 1]["out"] + bo
    return out


# revision 34
# speedup vs baseline: 1.3333x; 1.3333x over previous
"""Causal self-attention (B=4, S=2048, E=1024, H=16) on 8 trn2 NeuronCores.

Sharding: data parallel over batch (4) x tensor parallel over head groups (2).
Core c handles batch c//2, heads [ (c%2)*8, (c%2)*8+8 ).  Each core computes
its group's QKV projections, causal attention, and a partial output
projection; the host sums the two group partials per batch and adds bo.

All matmuls run in float32r (fp32 storage, single-pass relaxed-precision PE
mode — full bf16-rate at free dim >= 256, ~2^-10 product precision), with
fp32 PSUM accumulation.  Measured end-to-end error is ~10x tighter than a
bf16 datapath.

Device layout notes:
  - x arrives pre-transposed from host: xT [E, S], streamed per 512-token
    chunk.  QT/KT are produced feature-major [c, s] (lhsT = W, rhs = xT);
    V token-major [s, f] (lhsT = xT, rhs = Wv).
  - scores are computed transposed [k, q]: lhsT = KT head slice [64, 128],
    rhs = QT head slice [64, 512]; head parity selects partition base 0/64.
  - softmax denominator comes from a 65th ones-column in the AV stationary
    operand: PSUM row 64 of the [65, q] attention-value output is the
    running row-sum of exp(scores).  Its reciprocal (VectorE mid-pass,
    exp(-ln d) on ScalarE for latency-critical pass tails) is broadcast to
    64 partitions via a DRAM-bounce DMA.
  - causal masking: fully-masked k-tiles are skipped; on diagonal tiles the
    dead columns are zeroed on VectorE and the 128-wide triangle multiplied
    in on GpSimd, all off the PE critical path (AV matmuls lag 4 k-tiles).
"""

import numpy as np

import concourse.mybir as mybir
import concourse.tile as tile
from concourse import bacc
from concourse.bass_utils import run_bass_kernel_spmd

F32 = mybir.dt.float32
F32R = mybir.dt.float32r
Exp = mybir.ActivationFunctionType.Exp
MULT = mybir.AluOpType.mult
ADD = mybir.AluOpType.add

B, S, E, H = 4, 2048, 1024, 16
D = 64          # head dim
HG = 8          # heads per core
G = 512         # group feature width
P = 128
NKT = S // P    # 16 k-tiles
NST = S // P    # 16 s-tiles
QB = 512        # q-block width
NQB = S // QB   # 4
ESUB = E // P   # 8
VW = D + 1      # V stationary width (64 dims + ones column)

_CACHE = {}


def _build_program():
    nc = bacc.Bacc("TRN2", target_bir_lowering=False, debug=False)

    xt_d = nc.dram_tensor("xt", [E, S], F32R, kind="ExternalInput").ap()
    wq_d = nc.dram_tensor("wq", [E, G], F32R, kind="ExternalInput").ap()
    wk_d = nc.dram_tensor("wk", [E, G], F32R, kind="ExternalInput").ap()
    wv_d = nc.dram_tensor("wv", [E, G], F32R, kind="ExternalInput").ap()
    wo_d = nc.dram_tensor("wo", [G, E], F32R, kind="ExternalInput").ap()
    bq_d = nc.dram_tensor("bq", [P, 4], F32, kind="ExternalInput").ap()
    bk_d = nc.dram_tensor("bk", [P, 4], F32, kind="ExternalInput").ap()
    bv_d = nc.dram_tensor("bv", [P, G], F32, kind="ExternalInput").ap()
    tri_d = nc.dram_tensor("tri", [P, P], F32R, kind="ExternalInput").ap()
    one_d = nc.dram_tensor("one", [P, D], F32R, kind="ExternalInput").ap()
    out_d = nc.dram_tensor("out", [S, E], F32, kind="ExternalOutput").ap()
    # scratch for the reciprocal-row broadcast bounce
    rc_d = nc.dram_tensor("rc_scratch", [HG, NQB, QB], F32, kind="Internal").ap()

    qt_sb = nc.alloc_sbuf_tensor("qt_sb", [P, 4, S], F32R).ap()
    kt_sb = nc.alloc_sbuf_tensor("kt_sb", [P, 4, S], F32R).ap()
    vx_sb = nc.alloc_sbuf_tensor("vx_sb", [P, NKT, HG, VW], F32R).ap()
    tri_sb = nc.alloc_sbuf_tensor("tri_sb", [P, P], F32R).ap()
    ones_sb = nc.alloc_sbuf_tensor("ones_sb", [P, D], F32R).ap()
    bq_sb = nc.alloc_sbuf_tensor("bq_sb", [P, 4], F32).ap()
    bk_sb = nc.alloc_sbuf_tensor("bk_sb", [P, 4], F32).ap()
    bv_sb = nc.alloc_sbuf_tensor("bv_sb", [P, G], F32).ap()

    with tile.TileContext(nc) as tc:
        nc.sync.dma_start(bq_sb[:], bq_d[:])
        nc.sync.dma_start(bk_sb[:], bk_d[:])
        nc.sync.dma_start(bv_sb[:], bv_d[:])
        nc.sync.dma_start(tri_sb[:], tri_d[:])
        nc.sync.dma_start(ones_sb[:], one_d[:])

        xt_r = xt_d.rearrange("(o p) s -> p o s", p=P)

        # ---- QKV projections (xT streamed per 512-token chunk) ----
        with (
            tc.tile_pool(name="w_pool", bufs=1) as wp,
            tc.tile_pool(name="xt_pool", bufs=2) as xp,
            tc.tile_pool(name="proj_ps", bufs=2, space="PSUM") as pp,
        ):
            wqp = wp.tile([P, ESUB, G], F32R, tag="wq", name="wqp")
            wkp = wp.tile([P, ESUB, G], F32R, tag="wk", name="wkp")
            wvp = wp.tile([P, ESUB, G], F32R, tag="wv", name="wvp")
            # first xT chunk loads ahead of the weights so the PE starts sooner
            xtp0 = xp.tile([P, ESUB, QB], F32R, tag="xt", name="xtp")
            nc.sync.dma_start(xtp0[:], xt_r[:, :, 0:QB])
            for e in range(ESUB):
                nc.sync.dma_start(
                    wqp[:, e, :], wq_d.rearrange("(o p) f -> p o f", p=P)[:, e, :]
                )
                nc.sync.dma_start(
                    wkp[:, e, :], wk_d.rearrange("(o p) f -> p o f", p=P)[:, e, :]
                )
                nc.sync.dma_start(
                    wvp[:, e, :], wv_d.rearrange("(o p) f -> p o f", p=P)[:, e, :]
                )
            for qb in range(NQB):
                if qb == 0:
                    xtp = xtp0
                else:
                    xtp = xp.tile([P, ESUB, QB], F32R, tag="xt", name="xtp")
                    nc.sync.dma_start(xtp[:], xt_r[:, :, qb * QB : (qb + 1) * QB])
                for cc in range(4):
                    q_ps = pp.tile([P, QB], F32, tag="q", name="q_ps")
                    for e in range(ESUB):
                        nc.tensor.matmul(
                            q_ps[:],
                            lhsT=wqp[:, e, cc * P : (cc + 1) * P],
                            rhs=xtp[:, e, :],
                            start=(e == 0),
                            stop=(e == ESUB - 1),
                        )
                    # fold bias and the 1/sqrt(D) scale into Q
                    nc.vector.tensor_scalar(
                        qt_sb[:, cc, qb * QB : (qb + 1) * QB],
                        q_ps[:],
                        bq_sb[:, cc : cc + 1],
                        0.125,
                        ADD,
                        MULT,
                    )
                    k_ps = pp.tile([P, QB], F32, tag="k", name="k_ps")
                    for e in range(ESUB):
                        nc.tensor.matmul(
                            k_ps[:],
                            lhsT=wkp[:, e, cc * P : (cc + 1) * P],
                            rhs=xtp[:, e, :],
                            start=(e == 0),
                            stop=(e == ESUB - 1),
                        )
                    nc.vector.tensor_scalar_add(
                        kt_sb[:, cc, qb * QB : (qb + 1) * QB],
                        k_ps[:],
                        bk_sb[:, cc : cc + 1],
                    )
                for stl in range(4):
                    st = qb * 4 + stl
                    v_ps = pp.tile([P, G], F32, tag="v", name="v_ps")
                    for e in range(ESUB):
                        nc.tensor.matmul(
                            v_ps[:],
                            lhsT=xtp[:, e, stl * P : (stl + 1) * P],
                            rhs=wvp[:, e, :],
                            start=(e == 0),
                            stop=(e == ESUB - 1),
                        )
                    nc.vector.tensor_tensor(
                        vx_sb[:, st, :, 0:D],
                        v_ps.rearrange("p (h d) -> p h d", d=D),
                        bv_sb.rearrange("p (h d) -> p h d", d=D),
                        ADD,
                    )
                    nc.vector.tensor_copy(
                        vx_sb[:, st, :, D : D + 1],
                        ones_sb[:, 0:HG].rearrange("p (h u) -> p h u", u=1),
                    )

        # ---- attention + output projection ----
        with tc.tile_pool(name="at_pool", bufs=1) as atp:
            at_t = atp.tile([P, 4, S], F32R, name="at_t")
            # wo lives in the same long-lived pool and loads during attention
            wop = atp.tile([P, 4, E], F32R, name="wop")
            nc.sync.dma_start(wop[:], wo_d.rearrange("(o p) n -> p o n", p=P))
            with (
                tc.tile_pool(name="attn_ps", bufs=1, space="PSUM") as ap,
                tc.tile_pool(name="attn_sb", bufs=2) as sp,
            ):
                # heads run in even/odd pairs: the pair's score matmuls use
                # partition bases 0/64 (distinct PE row groups -> concurrent),
                # and the pair keeps the PE fed while ScalarE runs exp.
                # AV matmuls lag the scores by AV_LAG k-tiles so the PE never
                # waits on exp/mask; accumulators are per-(head, q-block) so
                # normalization runs mid-pass and frees PSUM slots early.
                AV_LAG = 4
                for pair in range(4):
                    heads = (2 * pair, 2 * pair + 1)
                    sub = pair

                    def normalize(h, qb):
                        hb = (h % 2) * D
                        a_ps = a_t[(h, qb)]
                        rs = sp.tile([VW, QB], F32, tag="rs", name="rs", bufs=2)
                        if qb % 2 == 0:
                            # mid-pass: latency-insensitive, VectorE has slack
                            nc.vector.reciprocal(rs[D:VW, :], a_ps[D:VW, :])
                        else:
                            # pass tail: 1/d = exp(-ln d) on ScalarE (shorter
                            # latency; ln+exp share one table set)
                            lg = sp.tile([VW, QB], F32, tag="lg", name="lg", bufs=2)
                            nc.scalar.activation(
                                lg[D:VW, :],
                                a_ps[D:VW, :],
                                mybir.ActivationFunctionType.Ln,
                            )
                            nc.scalar.activation(
                                rs[D:VW, :], lg[D:VW, :], Exp, scale=-1.0
                            )
                        # broadcast the reciprocal row to 64 partitions via a
                        # DRAM bounce (a DRAM source AP may repeat, SBUF cannot)
                        rb_sb = sp.tile([D, QB], F32, tag="rbs", name="rb_sb", bufs=2)
                        nc.sync.dma_start(rc_d[h, qb : qb + 1, :], rs[D:VW, :])
                        nc.sync.dma_start(
                            rb_sb[:], rc_d[h, qb : qb + 1, :].to_broadcast([D, QB])
                        )
                        at_slice = at_t[hb : hb + D, sub, qb * QB : (qb + 1) * QB]
                        if hb == 0:
                            nc.vector.tensor_tensor(
                                at_slice, a_ps[0:D, :], rb_sb[:], MULT
                            )
                        else:
                            tmp = sp.tile([D, QB], F32R, tag="tmp", name="tmp", bufs=2)
                            nc.vector.tensor_tensor(tmp[:], a_ps[0:D, :], rb_sb[:], MULT)
                            nc.sync.dma_start(at_slice, tmp[:])

                    for qp in range(2):
                        qb_lo, qb_hi = 2 * qp, 2 * qp + 1
                        a_t = {
                            (h, qb): ap.tile([VW, QB], F32, tag="a", name="a_ps", bufs=4)
                            for h in heads
                            for qb in (qb_lo, qb_hi)
                        }

                        def av_mms(pts_prev, kt_prev, qlo_prev):
                            for qb in range(qlo_prev, qb_hi + 1):
                                for h in heads:
                                    nc.tensor.matmul(
                                        a_t[(h, qb)][:],
                                        lhsT=vx_sb[:, kt_prev, h, :],
                                        rhs=pts_prev[qb][:, h % 2, :],
                                        start=(kt_prev == 0),
                                        stop=(kt_prev == 4 * qb + 3),
                                    )
                            for qb in range(qlo_prev, qb_hi + 1):
                                if kt_prev == 4 * qb + 3:
                                    for h in heads:
                                        normalize(h, qb)

                        pending = []
                        for kt in range(4 * qb_hi + 4):
                            qlo = max(qb_lo, kt // 4)
                            pts = {}
                            for qb in range(qlo, qb_hi + 1):
                                # one PSUM tile per (kt, qb): bank 0 = even
                                # head, bank 1 = odd head
                                s_t = ap.tile(
                                    [P, 2, QB], F32, tag="s", name="s_ps", bufs=2
                                )
                                for h in heads:
                                    hb = (h % 2) * D
                                    nc.tensor.matmul(
                                        s_t[:, h % 2, :],
                                        lhsT=kt_sb[
                                            hb : hb + D, sub, kt * P : (kt + 1) * P
                                        ],
                                        rhs=qt_sb[
                                            hb : hb + D, sub, qb * QB : (qb + 1) * QB
                                        ],
                                        start=True,
                                        stop=True,
                                    )
                                pt = sp.tile(
                                    [P, 2, QB], F32R, tag="pt", name="pt", bufs=9
                                )
                                nc.scalar.activation(pt[:], s_t[:], Exp)
                                if kt // 4 == qb:
                                    m = kt % 4
                                    for h in heads:
                                        if m > 0:
                                            nc.vector.tensor_scalar_mul(
                                                pt[:, h % 2, 0 : P * m],
                                                pt[:, h % 2, 0 : P * m],
                                                0.0,
                                            )
                                        nc.gpsimd.tensor_tensor(
                                            pt[:, h % 2, P * m : P * (m + 1)],
                                            pt[:, h % 2, P * m : P * (m + 1)],
                                            tri_sb[:],
                                            MULT,
                                        )
                                pts[qb] = pt
                            if len(pending) >= AV_LAG:
                                av_mms(*pending.pop(0))
                            pending.append((pts, kt, qlo))
                        for args in pending:
                            av_mms(*args)

            # ---- output projection (partial; host adds other group + bo) ----
            with (
                tc.tile_pool(name="op_ps", bufs=2, space="PSUM") as op,
                tc.tile_pool(name="op_sb", bufs=3) as osp,
            ):
                for st in range(NST):
                    for n in range(2):
                        o_ps = op.tile([P, QB], F32, tag="o", name="o_ps")
                        for t in range(4):
                            nc.tensor.matmul(
                                o_ps[:],
                                lhsT=at_t[:, t, st * P : (st + 1) * P],
                                rhs=wop[:, t, n * QB : (n + 1) * QB],
                                start=(t == 0),
                                stop=(t == 3),
                            )
                        o_sb = osp.tile([P, QB], F32, tag="ost", name="o_sb")
                        nc.vector.tensor_copy(o_sb[:], o_ps[:])
                        nc.sync.dma_start(
                            out_d[st * P : (st + 1) * P, n * QB : (n + 1) * QB],
                            o_sb[:],
                        )

    nc.compile()
    return nc


def _prep_inputs(x, Wqkv, bqkv, Wo, bo):
    x = np.asarray(x, np.float32)
    Wqkv = np.asarray(Wqkv, np.float32)
    bqkv = np.asarray(bqkv, np.float32)
    Wo = np.asarray(Wo, np.float32)

    # 128x128 inclusive lower-triangle-in-(q,k) == kl <= ql in [k, q] layout
    kl = np.arange(P)[:, None]
    tri = (kl <= np.arange(P)[None, :]).astype(np.float32)


    in_maps = []
    for c in range(8):
        b, g = divmod(c, 2)
        lo, hi = g * G, (g + 1) * G
        in_maps.append(
            {
                "xt": np.ascontiguousarray(x[b].T),
                "wq": np.ascontiguousarray(Wqkv[:, lo:hi]),
                "wk": np.ascontiguousarray(Wqkv[:, E + lo : E + hi]),
                "wv": np.ascontiguousarray(Wqkv[:, 2 * E + lo : 2 * E + hi]),
                "wo": np.ascontiguousarray(Wo[lo:hi, :]),
                "bq": np.ascontiguousarray(bqkv[lo:hi].reshape(4, P).T),
                "bk": np.ascontiguousarray(bqkv[E + lo : E + hi].reshape(4, P).T),
                "bv": np.tile(bqkv[2 * E + lo : 2 * E + hi][None, :], (P, 1)).astype(
                    np.float32
                ),
                "tri": tri,
                "one": np.ones((P, D), np.float32),
            }
        )
    return in_maps


def kernel(x, Wqkv, bqkv, Wo, bo, _trace=False):
    if "nc" not in _CACHE:
        _CACHE["nc"] = _build_program()
    nc = _CACHE["nc"]

    in_maps = _prep_inputs(x, Wqkv, bqkv, Wo, bo)
    res = run_bass_kernel_spmd(nc, in_maps, core_ids=list(range(8)), trace=_trace)
    _CACHE["last_result"] = res

    bo = np.asarray(bo, np.float32)
    out = np.empty((B, S, E), np.float32)
    for b in range(B):
        out[b] = res.results[2 * b]["out"] + res.results[2 * b + 1]["out"] + bo
    return out


# revision 35
# speedup vs baseline: 1.8546x; 1.3910x over previous
"""Causal self-attention (B=4, S=2048, E=1024, H=16) on 8 trn2 NeuronCores.

Sharding: data parallel over batch (4) x tensor parallel over head groups (2).
Core c handles batch c//2, heads [ (c%2)*8, (c%2)*8+8 ).  Each core computes
its group's QKV projections, causal attention, and a partial output
projection; the host sums the two group partials per batch and adds bo.

All matmuls run in float32r (fp32 storage, single-pass relaxed-precision PE
mode — full bf16-rate at free dim >= 256, ~2^-10 product precision), with
fp32 PSUM accumulation.  Measured end-to-end error is ~10x tighter than a
bf16 datapath.

Device layout notes:
  - x arrives pre-transposed from host: xT [E, S], streamed per 512-token
    chunk.  QT/KT are produced feature-major [c, s] (lhsT = W, rhs = xT);
    V token-major [s, f] (lhsT = xT, rhs = Wv).
  - scores are computed transposed [k, q]: lhsT = KT head slice [64, 128],
    rhs = QT head slice [64, 512]; head parity selects partition base 0/64.
  - softmax denominator comes from a 65th ones-column in the AV stationary
    operand: PSUM row 64 of the [65, q] attention-value output is the
    running row-sum of exp(scores).  Its reciprocal (VectorE mid-pass,
    exp(-ln d) on ScalarE for latency-critical pass tails) is broadcast to
    64 partitions via a DRAM-bounce DMA.
  - causal masking: fully-masked k-tiles are skipped; on diagonal tiles the
    dead columns are zeroed on VectorE and the 128-wide triangle multiplied
    in on GpSimd, all off the PE critical path (AV matmuls lag 4 k-tiles).
"""

import numpy as np

import concourse.mybir as mybir
import concourse.tile as tile
from concourse import bacc
from concourse.bass_utils import run_bass_kernel_spmd

F32 = mybir.dt.float32
F32R = mybir.dt.float32r
Exp = mybir.ActivationFunctionType.Exp
MULT = mybir.AluOpType.mult
ADD = mybir.AluOpType.add

B, S, E, H = 4, 2048, 1024, 16
D = 64          # head dim
HG = 8          # heads per core
G = 512         # group feature width
P = 128
NKT = S // P    # 16 k-tiles
NST = S // P    # 16 s-tiles
QB = 512        # q-block width
NQB = S // QB   # 4
ESUB = E // P   # 8
VW = D + 1      # V stationary width (64 dims + ones column)

_CACHE = {}


def _build_program():
    nc = bacc.Bacc("TRN2", target_bir_lowering=False, debug=False)

    xt_d = nc.dram_tensor("xt", [E, S], F32R, kind="ExternalInput").ap()
    wq_d = nc.dram_tensor("wq", [E, G], F32R, kind="ExternalInput").ap()
    wk_d = nc.dram_tensor("wk", [E, G], F32R, kind="ExternalInput").ap()
    wv_d = nc.dram_tensor("wv", [E, G], F32R, kind="ExternalInput").ap()
    wo_d = nc.dram_tensor("wo", [G, E], F32R, kind="ExternalInput").ap()
    bq_d = nc.dram_tensor("bq", [P, 4], F32, kind="ExternalInput").ap()
    bk_d = nc.dram_tensor("bk", [P, 4], F32, kind="ExternalInput").ap()
    bv_d = nc.dram_tensor("bv", [P, G], F32, kind="ExternalInput").ap()
    tri_d = nc.dram_tensor("tri", [P, P], F32R, kind="ExternalInput").ap()
    one_d = nc.dram_tensor("one", [P, D], F32R, kind="ExternalInput").ap()
    out_d = nc.dram_tensor("out", [S, E], F32, kind="ExternalOutput").ap()
    # scratch for the reciprocal-row broadcast bounce
    rc_d = nc.dram_tensor("rc_scratch", [HG, NQB, QB], F32, kind="Internal").ap()

    qt_sb = nc.alloc_sbuf_tensor("qt_sb", [P, 4, S], F32R).ap()
    kt_sb = nc.alloc_sbuf_tensor("kt_sb", [P, 4, S], F32R).ap()
    vx_sb = nc.alloc_sbuf_tensor("vx_sb", [P, NKT, HG, VW], F32R).ap()
    tri_sb = nc.alloc_sbuf_tensor("tri_sb", [P, P], F32R).ap()
    ones_sb = nc.alloc_sbuf_tensor("ones_sb", [P, D], F32R).ap()
    bq_sb = nc.alloc_sbuf_tensor("bq_sb", [P, 4], F32).ap()
    bk_sb = nc.alloc_sbuf_tensor("bk_sb", [P, 4], F32).ap()
    bv_sb = nc.alloc_sbuf_tensor("bv_sb", [P, G], F32).ap()

    with tile.TileContext(nc) as tc:
        nc.sync.dma_start(bq_sb[:], bq_d[:])
        nc.sync.dma_start(bk_sb[:], bk_d[:])
        nc.sync.dma_start(bv_sb[:], bv_d[:])
        nc.sync.dma_start(tri_sb[:], tri_d[:])
        nc.sync.dma_start(ones_sb[:], one_d[:])

        xt_r = xt_d.rearrange("(o p) s -> p o s", p=P)

        # ---- QKV projections (xT streamed per 512-token chunk) ----
        with (
            tc.tile_pool(name="w_pool", bufs=1) as wp,
            tc.tile_pool(name="xt_pool", bufs=2) as xp,
            tc.tile_pool(name="proj_ps", bufs=2, space="PSUM") as pp,
        ):
            wqp = wp.tile([P, ESUB, G], F32R, tag="wq", name="wqp")
            wkp = wp.tile([P, ESUB, G], F32R, tag="wk", name="wkp")
            wvp = wp.tile([P, ESUB, G], F32R, tag="wv", name="wvp")
            # first xT chunk loads ahead of the weights so the PE starts sooner
            xtp0 = xp.tile([P, ESUB, QB], F32R, tag="xt", name="xtp")
            nc.sync.dma_start(xtp0[:], xt_r[:, :, 0:QB])
            for e in range(ESUB):
                nc.sync.dma_start(
                    wqp[:, e, :], wq_d.rearrange("(o p) f -> p o f", p=P)[:, e, :]
                )
                nc.sync.dma_start(
                    wkp[:, e, :], wk_d.rearrange("(o p) f -> p o f", p=P)[:, e, :]
                )
                nc.sync.dma_start(
                    wvp[:, e, :], wv_d.rearrange("(o p) f -> p o f", p=P)[:, e, :]
                )
            for qb in range(NQB):
                if qb == 0:
                    xtp = xtp0
                else:
                    xtp = xp.tile([P, ESUB, QB], F32R, tag="xt", name="xtp")
                    nc.sync.dma_start(xtp[:], xt_r[:, :, qb * QB : (qb + 1) * QB])
                for cc in range(4):
                    q_ps = pp.tile([P, QB], F32, tag="q", name="q_ps")
                    for e in range(ESUB):
                        nc.tensor.matmul(
                            q_ps[:],
                            lhsT=wqp[:, e, cc * P : (cc + 1) * P],
                            rhs=xtp[:, e, :],
                            start=(e == 0),
                            stop=(e == ESUB - 1),
                        )
                    # fold bias and the 1/sqrt(D) scale into Q
                    nc.vector.tensor_scalar(
                        qt_sb[:, cc, qb * QB : (qb + 1) * QB],
                        q_ps[:],
                        bq_sb[:, cc : cc + 1],
                        0.125,
                        ADD,
                        MULT,
                    )
                    k_ps = pp.tile([P, QB], F32, tag="k", name="k_ps")
                    for e in range(ESUB):
                        nc.tensor.matmul(
                            k_ps[:],
                            lhsT=wkp[:, e, cc * P : (cc + 1) * P],
                            rhs=xtp[:, e, :],
                            start=(e == 0),
                            stop=(e == ESUB - 1),
                        )
                    nc.vector.tensor_scalar_add(
                        kt_sb[:, cc, qb * QB : (qb + 1) * QB],
                        k_ps[:],
                        bk_sb[:, cc : cc + 1],
                    )
                for stl in range(4):
                    st = qb * 4 + stl
                    v_ps = pp.tile([P, G], F32, tag="v", name="v_ps")
                    for e in range(ESUB):
                        nc.tensor.matmul(
                            v_ps[:],
                            lhsT=xtp[:, e, stl * P : (stl + 1) * P],
                            rhs=wvp[:, e, :],
                            start=(e == 0),
                            stop=(e == ESUB - 1),
                        )
                    nc.vector.tensor_tensor(
                        vx_sb[:, st, :, 0:D],
                        v_ps.rearrange("p (h d) -> p h d", d=D),
                        bv_sb.rearrange("p (h d) -> p h d", d=D),
                        ADD,
                    )
                    nc.vector.tensor_copy(
                        vx_sb[:, st, :, D : D + 1],
                        ones_sb[:, 0:HG].rearrange("p (h u) -> p h u", u=1),
                    )

        # ---- attention + output projection ----
        with tc.tile_pool(name="at_pool", bufs=1) as atp:
            at_t = atp.tile([P, 4, S], F32R, name="at_t")
            # wo lives in the same long-lived pool and loads during attention
            wop = atp.tile([P, 4, E], F32R, name="wop")
            nc.sync.dma_start(wop[:], wo_d.rearrange("(o p) n -> p o n", p=P))
            with (
                tc.tile_pool(name="attn_ps", bufs=1, space="PSUM") as ap,
                tc.tile_pool(name="attn_sb", bufs=2) as sp,
            ):
                # heads run in even/odd pairs: the pair's score matmuls use
                # partition bases 0/64 (distinct PE row groups -> concurrent),
                # and the pair keeps the PE fed while ScalarE runs exp.
                # AV matmuls lag the scores by AV_LAG k-tiles so the PE never
                # waits on exp/mask; accumulators are per-(head, q-block) so
                # normalization runs mid-pass and frees PSUM slots early.
                AV_LAG = 4
                for pair in range(4):
                    heads = (2 * pair, 2 * pair + 1)
                    sub = pair

                    def normalize(h, qb):
                        hb = (h % 2) * D
                        a_ps = a_t[(h, qb)]
                        # broadcast the RAW denominator row to 64 partitions
                        # via a DRAM bounce (a DRAM source AP may repeat, SBUF
                        # cannot), then take the reciprocal at partition base 0
                        # with the fast custom-DVE op (~3e-6 rel err; it
                        # mishandles nonzero partition bases, hence this order)
                        dn = sp.tile([VW, QB], F32, tag="lg", name="dn", bufs=2)
                        nc.vector.tensor_copy(dn[D:VW, :], a_ps[D:VW, :])
                        nc.sync.dma_start(rc_d[h, qb : qb + 1, :], dn[D:VW, :])
                        db = sp.tile([D, QB], F32, tag="rs", name="db", bufs=2)
                        nc.sync.dma_start(
                            db[:], rc_d[h, qb : qb + 1, :].to_broadcast([D, QB])
                        )
                        rb_sb = sp.tile([D, QB], F32, tag="rbs", name="rb_sb", bufs=2)
                        nc.vector.reciprocal_approx_fast(rb_sb[:], db[:])
                        at_slice = at_t[hb : hb + D, sub, qb * QB : (qb + 1) * QB]
                        if hb == 0:
                            nc.vector.tensor_tensor(
                                at_slice, a_ps[0:D, :], rb_sb[:], MULT
                            )
                        else:
                            tmp = sp.tile([D, QB], F32R, tag="tmp", name="tmp", bufs=2)
                            nc.vector.tensor_tensor(tmp[:], a_ps[0:D, :], rb_sb[:], MULT)
                            nc.sync.dma_start(at_slice, tmp[:])

                    for qp in range(2):
                        qb_lo, qb_hi = 2 * qp, 2 * qp + 1
                        a_t = {
                            (h, qb): ap.tile([VW, QB], F32, tag="a", name="a_ps", bufs=4)
                            for h in heads
                            for qb in (qb_lo, qb_hi)
                        }

                        def av_mms(pts_prev, kt_prev, qlo_prev):
                            for qb in range(qlo_prev, qb_hi + 1):
                                for h in heads:
                                    nc.tensor.matmul(
                                        a_t[(h, qb)][:],
                                        lhsT=vx_sb[:, kt_prev, h, :],
                                        rhs=pts_prev[qb][:, h % 2, :],
                                        start=(kt_prev == 0),
                                        stop=(kt_prev == 4 * qb + 3),
                                    )
                            for qb in range(qlo_prev, qb_hi + 1):
                                if kt_prev == 4 * qb + 3:
                                    for h in heads:
                                        normalize(h, qb)

                        pending = []
                        for kt in range(4 * qb_hi + 4):
                            qlo = max(qb_lo, kt // 4)
                            pts = {}
                            for qb in range(qlo, qb_hi + 1):
                                # one PSUM tile per (kt, qb): bank 0 = even
                                # head, bank 1 = odd head
                                s_t = ap.tile(
                                    [P, 2, QB], F32, tag="s", name="s_ps", bufs=2
                                )
                                for h in heads:
                                    hb = (h % 2) * D
                                    nc.tensor.matmul(
                                        s_t[:, h % 2, :],
                                        lhsT=kt_sb[
                                            hb : hb + D, sub, kt * P : (kt + 1) * P
                                        ],
                                        rhs=qt_sb[
                                            hb : hb + D, sub, qb * QB : (qb + 1) * QB
                                        ],
                                        start=True,
                                        stop=True,
                                    )
                                pt = sp.tile(
                                    [P, 2, QB], F32R, tag="pt", name="pt", bufs=9
                                )
                                nc.scalar.activation(pt[:], s_t[:], Exp)
                                if kt // 4 == qb:
                                    m = kt % 4
                                    for h in heads:
                                        if m > 0:
                                            nc.vector.tensor_scalar_mul(
                                                pt[:, h % 2, 0 : P * m],
                                                pt[:, h % 2, 0 : P * m],
                                                0.0,
                                            )
                                        nc.gpsimd.tensor_tensor(
                                            pt[:, h % 2, P * m : P * (m + 1)],
                                            pt[:, h % 2, P * m : P * (m + 1)],
                                            tri_sb[:],
                                            MULT,
                                        )
                                pts[qb] = pt
                            if len(pending) >= AV_LAG:
                                av_mms(*pending.pop(0))
                            pending.append((pts, kt, qlo))
                        for args in pending:
                            av_mms(*args)

            # ---- output projection (partial; host adds other group + bo) ----
            with (
                tc.tile_pool(name="op_ps", bufs=2, space="PSUM") as op,
                tc.tile_pool(name="op_sb", bufs=3) as osp,
            ):
                for st in range(NST):
                    for n in range(2):
                        o_ps = op.tile([P, QB], F32, tag="o", name="o_ps")
                        for t in range(4):
                            nc.tensor.matmul(
                                o_ps[:],
                                lhsT=at_t[:, t, st * P : (st + 1) * P],
                                rhs=wop[:, t, n * QB : (n + 1) * QB],
                                start=(t == 0),
                                stop=(t == 3),
                            )
                        o_sb = osp.tile([P, QB], F32, tag="ost", name="o_sb")
                        nc.vector.tensor_copy(o_sb[:], o_ps[:])
                        nc.sync.dma_start(
                            out_d[st * P : (st + 1) * P, n * QB : (n + 1) * QB],
                            o_sb[:],
                        )

    nc.compile()
    return nc


def _prep_inputs(x, Wqkv, bqkv, Wo, bo):
    x = np.asarray(x, np.float32)
    Wqkv = np.asarray(Wqkv, np.float32)
    bqkv = np.asarray(bqkv, np.float32)
    Wo = np.asarray(Wo, np.float32)

    # 128x128 inclusive lower-triangle-in-(q,k) == kl <= ql in [k, q] layout
    kl = np.arange(P)[:, None]
    tri = (kl <= np.arange(P)[None, :]).astype(np.float32)


    in_maps = []
    for c in range(8):
        b, g = divmod(c, 2)
        lo, hi = g * G, (g + 1) * G
        in_maps.append(
            {
                "xt": np.ascontiguousarray(x[b].T),
                "wq": np.ascontiguousarray(Wqkv[:, lo:hi]),
                "wk": np.ascontiguousarray(Wqkv[:, E + lo : E + hi]),
                "wv": np.ascontiguousarray(Wqkv[:, 2 * E + lo : 2 * E + hi]),
                "wo": np.ascontiguousarray(Wo[lo:hi, :]),
                "bq": np.ascontiguousarray(bqkv[lo:hi].reshape(4, P).T),
                "bk": np.ascontiguousarray(bqkv[E + lo : E + hi].reshape(4, P).T),
                "bv": np.tile(bqkv[2 * E + lo : 2 * E + hi][None, :], (P, 1)).astype(
                    np.float32
                ),
                "tri": tri,
                "one": np.ones((P, D), np.float32),
            }
        )
    return in_maps


def kernel(x, Wqkv, bqkv, Wo, bo, _trace=False):
    if "nc" not in _CACHE:
        _CACHE["nc"] = _build_program()
    nc = _CACHE["nc"]

    in_maps = _prep_inputs(x, Wqkv, bqkv, Wo, bo)
    res = run_bass_kernel_spmd(nc, in_maps, core_ids=list(range(8)), trace=_trace)
    _CACHE["last_result"] = res

    bo = np.asarray(bo, np.float32)
    out = np.empty((B, S, E), np.float32)
    for b in range(B):
        out[b] = res.results[2 * b]["out"] + res.results[2 * b + 1]["out"] + bo
    return out


# revision 37
# speedup vs baseline: 1.8884x; 1.0182x over previous
"""Causal self-attention (B=4, S=2048, E=1024, H=16) on 8 trn2 NeuronCores.

Sharding: data parallel over batch (4) x tensor parallel over head groups (2).
Core c handles batch c//2, heads [ (c%2)*8, (c%2)*8+8 ).  Each core computes
its group's QKV projections, causal attention, and a partial output
projection; the host sums the two group partials per batch and adds bo.

All matmuls run in float32r (fp32 storage, single-pass relaxed-precision PE
mode — full bf16-rate at free dim >= 256, ~2^-10 product precision), with
fp32 PSUM accumulation.  Measured end-to-end error is ~10x tighter than a
bf16 datapath.

Device layout notes:
  - x arrives pre-transposed from host: xT [E, S], streamed per 512-token
    chunk.  QT/KT are produced feature-major [c, s] (lhsT = W, rhs = xT);
    V token-major [s, f] (lhsT = xT, rhs = Wv).
  - scores are computed transposed [k, q]: lhsT = KT head slice [64, 128],
    rhs = QT head slice [64, 512]; head parity selects partition base 0/64.
  - softmax denominator comes from a 65th ones-column in the AV stationary
    operand: PSUM row 64 of the [65, q] attention-value output is the
    running row-sum of exp(scores).  The raw row is broadcast to 64
    partitions via a DRAM-bounce DMA, then inverted at partition base 0
    with the single-op custom-DVE fast reciprocal (~3e-6 rel err).
  - causal masking: fully-masked k-tiles are skipped; on diagonal tiles the
    dead columns are zeroed on VectorE and the 128-wide triangle multiplied
    in on GpSimd, all off the PE critical path (AV matmuls lag 4 k-tiles).
"""

import numpy as np

import concourse.mybir as mybir
import concourse.tile as tile
from concourse import bacc
from concourse.bass_utils import run_bass_kernel_spmd

F32 = mybir.dt.float32
F32R = mybir.dt.float32r
Exp = mybir.ActivationFunctionType.Exp
MULT = mybir.AluOpType.mult
ADD = mybir.AluOpType.add

B, S, E, H = 4, 2048, 1024, 16
D = 64          # head dim
HG = 8          # heads per core
G = 512         # group feature width
P = 128
NKT = S // P    # 16 k-tiles
NST = S // P    # 16 s-tiles
QB = 512        # q-block width
NQB = S // QB   # 4
ESUB = E // P   # 8
VW = D + 1      # V stationary width (64 dims + ones column)

_CACHE = {}


def _build_program():
    nc = bacc.Bacc("TRN2", target_bir_lowering=False, debug=False)

    xt_d = nc.dram_tensor("xt", [E, S], F32R, kind="ExternalInput").ap()
    wq_d = nc.dram_tensor("wq", [E, G], F32R, kind="ExternalInput").ap()
    wk_d = nc.dram_tensor("wk", [E, G], F32R, kind="ExternalInput").ap()
    wv_d = nc.dram_tensor("wv", [E, G], F32R, kind="ExternalInput").ap()
    wo_d = nc.dram_tensor("wo", [G, E], F32R, kind="ExternalInput").ap()
    bq_d = nc.dram_tensor("bq", [P, 4], F32, kind="ExternalInput").ap()
    bk_d = nc.dram_tensor("bk", [P, 4], F32, kind="ExternalInput").ap()
    bv_d = nc.dram_tensor("bv", [P, G], F32, kind="ExternalInput").ap()
    tri_d = nc.dram_tensor("tri", [P, P], F32R, kind="ExternalInput").ap()
    one_d = nc.dram_tensor("one", [P, D], F32R, kind="ExternalInput").ap()
    out_d = nc.dram_tensor("out", [S, E], F32, kind="ExternalOutput").ap()
    # scratch for the reciprocal-row broadcast bounce
    rc_d = nc.dram_tensor("rc_scratch", [HG, NQB, QB], F32, kind="Internal").ap()

    qt_sb = nc.alloc_sbuf_tensor("qt_sb", [P, 4, S], F32R).ap()
    kt_sb = nc.alloc_sbuf_tensor("kt_sb", [P, 4, S], F32R).ap()
    vx_sb = nc.alloc_sbuf_tensor("vx_sb", [P, NKT, HG, VW], F32R).ap()
    tri_sb = nc.alloc_sbuf_tensor("tri_sb", [P, P], F32R).ap()
    ones_sb = nc.alloc_sbuf_tensor("ones_sb", [P, D], F32R).ap()
    bq_sb = nc.alloc_sbuf_tensor("bq_sb", [P, 4], F32).ap()
    bk_sb = nc.alloc_sbuf_tensor("bk_sb", [P, 4], F32).ap()
    bv_sb = nc.alloc_sbuf_tensor("bv_sb", [P, G], F32).ap()

    with tile.TileContext(nc) as tc:
        nc.sync.dma_start(bq_sb[:], bq_d[:])
        nc.sync.dma_start(bk_sb[:], bk_d[:])
        nc.sync.dma_start(bv_sb[:], bv_d[:])
        nc.sync.dma_start(tri_sb[:], tri_d[:])
        nc.sync.dma_start(ones_sb[:], one_d[:])

        xt_r = xt_d.rearrange("(o p) s -> p o s", p=P)

        # ---- QKV projections (xT streamed per 512-token chunk) ----
        with (
            tc.tile_pool(name="w_pool", bufs=1) as wp,
            tc.tile_pool(name="xt_pool", bufs=2) as xp,
            tc.tile_pool(name="proj_ps", bufs=2, space="PSUM") as pp,
        ):
            wqp = wp.tile([P, ESUB, G], F32R, tag="wq", name="wqp")
            wkp = wp.tile([P, ESUB, G], F32R, tag="wk", name="wkp")
            wvp = wp.tile([P, ESUB, G], F32R, tag="wv", name="wvp")
            # first xT chunk loads ahead of the weights so the PE starts sooner
            xtp0 = xp.tile([P, ESUB, QB], F32R, tag="xt", name="xtp")
            nc.sync.dma_start(xtp0[:], xt_r[:, :, 0:QB])
            for e in range(ESUB):
                nc.sync.dma_start(
                    wqp[:, e, :], wq_d.rearrange("(o p) f -> p o f", p=P)[:, e, :]
                )
                nc.sync.dma_start(
                    wkp[:, e, :], wk_d.rearrange("(o p) f -> p o f", p=P)[:, e, :]
                )
                nc.sync.dma_start(
                    wvp[:, e, :], wv_d.rearrange("(o p) f -> p o f", p=P)[:, e, :]
                )
            for qb in range(NQB):
                if qb == 0:
                    xtp = xtp0
                else:
                    xtp = xp.tile([P, ESUB, QB], F32R, tag="xt", name="xtp")
                    nc.sync.dma_start(xtp[:], xt_r[:, :, qb * QB : (qb + 1) * QB])
                for cc in range(4):
                    q_ps = pp.tile([P, QB], F32, tag="q", name="q_ps")
                    for e in range(ESUB):
                        nc.tensor.matmul(
                            q_ps[:],
                            lhsT=wqp[:, e, cc * P : (cc + 1) * P],
                            rhs=xtp[:, e, :],
                            start=(e == 0),
                            stop=(e == ESUB - 1),
                        )
                    # fold bias and the 1/sqrt(D) scale into Q
                    nc.vector.tensor_scalar(
                        qt_sb[:, cc, qb * QB : (qb + 1) * QB],
                        q_ps[:],
                        bq_sb[:, cc : cc + 1],
                        0.125,
                        ADD,
                        MULT,
                    )
                    k_ps = pp.tile([P, QB], F32, tag="k", name="k_ps")
                    for e in range(ESUB):
                        nc.tensor.matmul(
                            k_ps[:],
                            lhsT=wkp[:, e, cc * P : (cc + 1) * P],
                            rhs=xtp[:, e, :],
                            start=(e == 0),
                            stop=(e == ESUB - 1),
                        )
                    nc.vector.tensor_scalar_add(
                        kt_sb[:, cc, qb * QB : (qb + 1) * QB],
                        k_ps[:],
                        bk_sb[:, cc : cc + 1],
                    )
                for stl in range(4):
                    st = qb * 4 + stl
                    v_ps = pp.tile([P, G], F32, tag="v", name="v_ps")
                    for e in range(ESUB):
                        nc.tensor.matmul(
                            v_ps[:],
                            lhsT=xtp[:, e, stl * P : (stl + 1) * P],
                            rhs=wvp[:, e, :],
                            start=(e == 0),
                            stop=(e == ESUB - 1),
                        )
                    nc.vector.tensor_tensor(
                        vx_sb[:, st, :, 0:D],
                        v_ps.rearrange("p (h d) -> p h d", d=D),
                        bv_sb.rearrange("p (h d) -> p h d", d=D),
                        ADD,
                    )
                    nc.vector.tensor_copy(
                        vx_sb[:, st, :, D : D + 1],
                        ones_sb[:, 0:HG].rearrange("p (h u) -> p h u", u=1),
                    )

        # ---- attention + output projection ----
        with tc.tile_pool(name="at_pool", bufs=1) as atp:
            at_t = atp.tile([P, 4, S], F32R, name="at_t")
            # wo lives in the same long-lived pool and loads during attention
            wop = atp.tile([P, 4, E], F32R, name="wop")
            nc.sync.dma_start(wop[:], wo_d.rearrange("(o p) n -> p o n", p=P))
            with (
                tc.tile_pool(name="attn_ps", bufs=1, space="PSUM") as ap,
                tc.tile_pool(name="attn_sb", bufs=2) as sp,
            ):
                # heads run in even/odd pairs: the pair's score matmuls use
                # partition bases 0/64 (distinct PE row groups -> concurrent),
                # and the pair keeps the PE fed while ScalarE runs exp.
                # AV matmuls lag the scores by AV_LAG k-tiles so the PE never
                # waits on exp/mask; accumulators are per-(head, q-block) so
                # normalization runs mid-pass and frees PSUM slots early.
                AV_LAG = 4
                for pair in range(4):
                    heads = (2 * pair, 2 * pair + 1)
                    sub = pair

                    def normalize(h, qb):
                        hb = (h % 2) * D
                        a_ps = a_t[(h, qb)]
                        # broadcast the RAW denominator row to 64 partitions
                        # via a DRAM bounce (a DRAM source AP may repeat, SBUF
                        # cannot), then take the reciprocal at partition base 0
                        # with the fast custom-DVE op (~3e-6 rel err; it
                        # mishandles nonzero partition bases, hence this order)
                        dn = sp.tile([VW, QB], F32, tag="lg", name="dn", bufs=2)
                        nc.vector.tensor_copy(dn[D:VW, :], a_ps[D:VW, :])
                        nc.sync.dma_start(rc_d[h, qb : qb + 1, :], dn[D:VW, :])
                        db = sp.tile([D, QB], F32, tag="rs", name="db", bufs=2)
                        nc.sync.dma_start(
                            db[:], rc_d[h, qb : qb + 1, :].to_broadcast([D, QB])
                        )
                        rb_sb = sp.tile([D, QB], F32, tag="rbs", name="rb_sb", bufs=2)
                        nc.vector.reciprocal_approx_fast(rb_sb[:], db[:])
                        at_slice = at_t[hb : hb + D, sub, qb * QB : (qb + 1) * QB]
                        if hb == 0:
                            nc.vector.tensor_tensor(
                                at_slice, a_ps[0:D, :], rb_sb[:], MULT
                            )
                        else:
                            tmp = sp.tile([D, QB], F32R, tag="tmp", name="tmp", bufs=2)
                            nc.vector.tensor_tensor(tmp[:], a_ps[0:D, :], rb_sb[:], MULT)
                            nc.sync.dma_start(at_slice, tmp[:])

                    for qp in range(2):
                        qb_lo, qb_hi = 2 * qp, 2 * qp + 1
                        a_t = {
                            (h, qb): ap.tile([VW, QB], F32, tag="a", name="a_ps", bufs=4)
                            for h in heads
                            for qb in (qb_lo, qb_hi)
                        }

                        def av_mms(pts_prev, kt_prev, qlo_prev):
                            for qb in range(qlo_prev, qb_hi + 1):
                                for h in heads:
                                    nc.tensor.matmul(
                                        a_t[(h, qb)][:],
                                        lhsT=vx_sb[:, kt_prev, h, :],
                                        rhs=pts_prev[qb][:, h % 2, :],
                                        start=(kt_prev == 0),
                                        stop=(kt_prev == 4 * qb + 3),
                                    )
                            for qb in range(qlo_prev, qb_hi + 1):
                                if kt_prev == 4 * qb + 3:
                                    for h in heads:
                                        normalize(h, qb)

                        pending = []
                        for kt in range(4 * qb_hi + 4):
                            qlo = max(qb_lo, kt // 4)
                            pts = {}
                            for qb in range(qlo, qb_hi + 1):
                                # one PSUM tile per (kt, qb): bank 0 = even
                                # head, bank 1 = odd head
                                s_t = ap.tile(
                                    [P, 2, QB], F32, tag="s", name="s_ps", bufs=2
                                )
                                for h in heads:
                                    hb = (h % 2) * D
                                    nc.tensor.matmul(
                                        s_t[:, h % 2, :],
                                        lhsT=kt_sb[
                                            hb : hb + D, sub, kt * P : (kt + 1) * P
                                        ],
                                        rhs=qt_sb[
                                            hb : hb + D, sub, qb * QB : (qb + 1) * QB
                                        ],
                                        start=True,
                                        stop=True,
                                    )
                                pt = sp.tile(
                                    [P, 2, QB], F32R, tag="pt", name="pt", bufs=9
                                )
                                if kt // 4 == qb and kt % 4 > 0:
                                    # diagonal tile: skip exp on the dead left
                                    # columns (zeroed below on VectorE)
                                    m = kt % 4
                                    nc.scalar.activation(
                                        pt[:, :, P * m :], s_t[:, :, P * m :], Exp
                                    )
                                else:
                                    nc.scalar.activation(pt[:], s_t[:], Exp)
                                if kt // 4 == qb:
                                    m = kt % 4
                                    for h in heads:
                                        if m > 0:
                                            nc.vector.tensor_scalar_mul(
                                                pt[:, h % 2, 0 : P * m],
                                                pt[:, h % 2, 0 : P * m],
                                                0.0,
                                            )
                                        nc.gpsimd.tensor_tensor(
                                            pt[:, h % 2, P * m : P * (m + 1)],
                                            pt[:, h % 2, P * m : P * (m + 1)],
                                            tri_sb[:],
                                            MULT,
                                        )
                                pts[qb] = pt
                            if len(pending) >= AV_LAG:
                                av_mms(*pending.pop(0))
                            pending.append((pts, kt, qlo))
                        for args in pending:
                            av_mms(*args)

            # ---- output projection (partial; host adds other group + bo) ----
            with (
                tc.tile_pool(name="op_ps", bufs=2, space="PSUM") as op,
                tc.tile_pool(name="op_sb", bufs=3) as osp,
            ):
                for st in range(NST):
                    for n in range(2):
                        o_ps = op.tile([P, QB], F32, tag="o", name="o_ps")
                        for t in range(4):
                            nc.tensor.matmul(
                                o_ps[:],
                                lhsT=at_t[:, t, st * P : (st + 1) * P],
                                rhs=wop[:, t, n * QB : (n + 1) * QB],
                                start=(t == 0),
                                stop=(t == 3),
                            )
                        o_sb = osp.tile([P, QB], F32, tag="ost", name="o_sb")
                        nc.vector.tensor_copy(o_sb[:], o_ps[:])
                        nc.sync.dma_start(
                            out_d[st * P : (st + 1) * P, n * QB : (n + 1) * QB],
                            o_sb[:],
                        )

    nc.compile()
    return nc


def _prep_inputs(x, Wqkv, bqkv, Wo, bo):
    x = np.asarray(x, np.float32)
    Wqkv = np.asarray(Wqkv, np.float32)
    bqkv = np.asarray(bqkv, np.float32)
    Wo = np.asarray(Wo, np.float32)

    # 128x128 inclusive lower-triangle-in-(q,k) == kl <= ql in [k, q] layout
    kl = np.arange(P)[:, None]
    tri = (kl <= np.arange(P)[None, :]).astype(np.float32)


    in_maps = []
    for c in range(8):
        b, g = divmod(c, 2)
        lo, hi = g * G, (g + 1) * G
        in_maps.append(
            {
                "xt": np.ascontiguousarray(x[b].T),
                "wq": np.ascontiguousarray(Wqkv[:, lo:hi]),
                "wk": np.ascontiguousarray(Wqkv[:, E + lo : E + hi]),
                "wv": np.ascontiguousarray(Wqkv[:, 2 * E + lo : 2 * E + hi]),
                "wo": np.ascontiguousarray(Wo[lo:hi, :]),
                "bq": np.ascontiguousarray(bqkv[lo:hi].reshape(4, P).T),
                "bk": np.ascontiguousarray(bqkv[E + lo : E + hi].reshape(4, P).T),
                "bv": np.tile(bqkv[2 * E + lo : 2 * E + hi][None, :], (P, 1)).astype(
                    np.float32
                ),
                "tri": tri,
                "one": np.ones((P, D), np.float32),
            }
        )
    return in_maps


def kernel(x, Wqkv, bqkv, Wo, bo, _trace=False):
    if "nc" not in _CACHE:
        _CACHE["nc"] = _build_program()
    nc = _CACHE["nc"]

    in_maps = _prep_inputs(x, Wqkv, bqkv, Wo, bo)
    res = run_bass_kernel_spmd(nc, in_maps, core_ids=list(range(8)), trace=_trace)
    _CACHE["last_result"] = res

    bo = np.asarray(bo, np.float32)
    out = np.empty((B, S, E), np.float32)
    for b in range(B):
        out[b] = res.results[2 * b]["out"] + res.results[2 * b + 1]["out"] + bo
    return out
